# revision 1
# baseline (speedup 1.0000x reference)
"""Trainium2 Bass kernel for nn_AdjacencyMatrix (gnn_message_passing).

Math: the reference keeps state = W * v[:, None] at all times, where
  v0 = pad(x[0], n),  v_{t+1} = W^T v_t  (colsum of state),
and the output is diag(W)[-256:] * v_num_steps[-256:].

So the whole problem collapses to num_steps matvecs v <- W^T v plus an
elementwise multiply by the last 256 diagonal entries of W.  Step 1 only
needs rows 0:1024 of W (v0 is zero elsewhere); the last step only needs
the last 256 columns of W.

Sharding (8 cores): row-parallel. Core d owns rows r_d = [1024d, 1024d+1024).
 - v1[r_d] = W[0:1024, r_d]^T x                      (local, no collective)
 - middle steps: partial = W[r_d,:]^T v[r_d] -> AllToAll + local 8-way sum
   gives core d exactly v_next[r_d]  (A2A is far cheaper than RS/AR on 8
   LNC1 ranks: mesh-style vs a 14-step ring)
 - final step: partial256 = W[r_d, -256:]^T v[r_d] -> AllReduce
 - out = diag * v_last[-256:]                        (identical on all cores)

W is stored/streamed as bf16 (vector math stays fp32): halves HBM traffic
and the whole 16 MiB row-shard stays SBUF-resident, so the second middle
pass re-reads nothing.

Layout convention: per-core vectors live in SBUF as [128, 8] tiles with
(p, k) = v[1024d + 8p + k]; W k-tile k holds rows {8p + k} (strided DMA),
so collective results load/sum directly into that layout with zero
transposes.
"""

import ml_dtypes
import numpy as np

import concourse.bass as bass
import concourse.mybir as mybir
from concourse import bacc, tile
from concourse.bass_utils import run_bass_kernel_spmd

N = 8192
SEG = 256 // 8            # 32 output elements per core
IN_N = 1024
OUT_N = 256
NCORES = 8
RP = N // NCORES          # rows per core = 1024
KT = RP // 128            # k-tiles per core = 8
D0 = N - OUT_N            # 7936

F32 = mybir.dt.float32
BF16 = mybir.dt.bfloat16
RG = [list(range(NCORES))]

PANEL = 2048              # W columns per DMA panel (compute granularity)
CH = 512                  # psum chunk (one fp32 PSUM bank)
HALF = N // 2             # A2A half size (host-permuted columns)

_cache: dict = {}


def _build(num_steps: int):
    """Build + compile the SPMD graph for num_steps >= 2."""
    n_mid = num_steps - 2
    nc = bacc.Bacc(
        "TRN2", target_bir_lowering=False, debug=False, num_devices=NCORES
    )
    xT = nc.declare_dram_parameter("xT", [128, 8], BF16, isOutput=False)
    A = nc.declare_dram_parameter("A", [IN_N, RP], BF16, isOutput=False)
    Wr = nc.declare_dram_parameter("Wr", [N // PANEL * KT, 128, PANEL], BF16, isOutput=False)
    Wc = nc.declare_dram_parameter("Wc", [KT, 128, OUT_N], BF16, isOutput=False)
    dg = nc.declare_dram_parameter("dg", [1, SEG], F32, isOutput=False)
    out = nc.declare_dram_parameter("out", [1, SEG], F32, isOutput=True)



    with tile.TileContext(nc) as tc:
        with (
            tc.tile_pool(name="small", bufs=1) as small,
            tc.tile_pool(name="apool", bufs=1) as apool,
            tc.tile_pool(name="wres", bufs=1) as wres,
            tc.tile_pool(name="ppool", bufs=6, space="PSUM") as ppool,
            tc.tile_pool(name="pp1", bufs=1, space="PSUM") as pp1,
            tc.tile_pool(name="dram", bufs=1, space="DRAM") as dram,
        ):
            # ---------------- stage 1: u1 = A^T x (local v1 slice) ----------
            xt = small.tile([128, 8], BF16, name="xt")
            nc.scalar.dma_start(out=xt[:, :], in_=xT.ap())
            a_sb = apool.tile([128, KT * RP], BF16, name="a_sb")
            for k in range(KT):
                nc.scalar.dma_start(
                    out=a_sb[:, k * RP:(k + 1) * RP],
                    in_=A.ap()[k * 128:(k + 1) * 128, :],
                )
            u1_ps = pp1.tile([128, 8], F32, name="u1_ps", tag="aux", bufs=2)
            for m in range(8):
                for k in range(KT):
                    nc.tensor.matmul(
                        u1_ps[:, m:m + 1],
                        lhsT=a_sb[:, k * RP + m * 128: k * RP + (m + 1) * 128],
                        rhs=xt[:, k:k + 1],
                        start=(k == 0),
                        stop=(k == KT - 1),
                    )
            u_cur = small.tile([128, 8], BF16, name="u1")
            nc.vector.tensor_copy(u_cur[:, :], u1_ps[:, :])

            partial = small.tile([1, N], F32, name="partial")
            ones8 = small.tile([8, 1], F32, name="ones8")
            nc.vector.memset(ones8[0:8, :], 1.0)

            # resident W row-shard: 8 k-tiles of [128, 8192] bf16 (16 MiB)
            wk = [
                wres.tile([128, N], BF16, name=f"wk_{k}") for k in range(KT)
            ]
            # early small prefetches for the final stage
            wc = small.tile([128, KT * OUT_N], BF16, name="wc")
            for k in range(KT):
                nc.scalar.dma_start(
                    out=wc[:, k * OUT_N:(k + 1) * OUT_N], in_=Wc.ap()[k]
                )
            dgt = small.tile([1, SEG], F32, name="dgt")
            nc.scalar.dma_start(out=dgt[0:1, :], in_=dg.ap())

            # ---------------- middle steps (num_steps - 2 of them) ----------
            for s in range(n_mid):
                un_ps = pp1.tile(
                    [128, 8], F32, name=f"un_ps_{s}", tag="aux", bufs=2
                )
                for h in range(2):     # half h = permuted cols [4096h, +4096)
                    for j in range(2 * h, 2 * h + 2):
                        if s == 0:
                            # panel-major DMA into the resident k-tile tiles
                            for k in range(KT):
                                nc.sync.dma_start(
                                    out=wk[k][:, j * PANEL:(j + 1) * PANEL],
                                    in_=Wr.ap()[j * KT + k],
                                )
                        nch = PANEL // CH
                        pss = [
                            ppool.tile(
                                [1, CH], F32, name=f"ps_{s}_{j}_{c}", tag="ps"
                            )
                            for c in range(nch)
                        ]
                        for k in range(KT):
                            for c in range(nch):
                                nc.tensor.matmul(
                                    pss[c][0:1, :],
                                    lhsT=u_cur[:, k:k + 1],
                                    rhs=wk[k][:, j * PANEL + c * CH:
                                              j * PANEL + (c + 1) * CH],
                                    start=(k == 0),
                                    stop=(k == KT - 1),
                                )
                        for c in range(nch):
                            col = j * PANEL + c * CH
                            nc.scalar.copy(
                                out=partial[0:1, col:col + CH],
                                in_=pss[c][0:1, :],
                            )
                    # AllToAll this half (512 per peer) + 8-way sum into
                    # partitions [64h, 64h+64) of un_ps.
                    cc_in = dram.tile([1, HALF], F32, name=f"cc_in_{s}_{h}")
                    cc_out = dram.tile(
                        [NCORES, HALF // NCORES], F32, name=f"cc_out_{s}_{h}"
                    )
                    nc.gpsimd.dma_start(
                        out=cc_in[:, :],
                        in_=partial[0:1, h * HALF:(h + 1) * HALF],
                    )
                    nc.gpsimd.collective_compute(
                        "AllToAll",
                        mybir.AluOpType.bypass,
                        replica_groups=RG,
                        ins=[cc_in.opt()],
                        outs=[cc_out.opt()],
                    )
                    if h == 1:
                        # keep the PE busy through the A2A wait so HAM stays
                        # at full clock for the next pass (result discarded
                        # into partial[0:CH], which is dead at this point)
                        wm = ppool.tile(
                            [1, CH], F32, name=f"wm_{s}", tag="ps"
                        )
                        for wi in range(40):
                            nc.tensor.matmul(
                                wm[0:1, :],
                                lhsT=u_cur[:, 0:1],
                                rhs=wk[wi % KT][:, 0:CH],
                                start=(wi == 0),
                                stop=(wi == 39),
                            )
                        nc.scalar.copy(
                            out=partial[0:1, 0:CH], in_=wm[0:1, :]
                        )
                    acc = small.tile(
                        [NCORES, HALF // NCORES], F32,
                        name=f"acc_{s}_{h}", tag="acc",
                    )
                    nc.sync.dma_start(out=acc[0:NCORES, :], in_=cc_out[:, :])
                    acc3 = acc[0:NCORES, :].rearrange("s (p k) -> k s p", k=8)
                    for k in range(8):
                        nc.tensor.matmul(
                            un_ps[64 * h:64 * h + 64, k:k + 1],
                            lhsT=acc3[k],
                            rhs=ones8[0:NCORES, 0:1],
                            start=True,
                            stop=True,
                        )
                u_next = small.tile([128, 8], BF16, name=f"u_{s + 2}")
                nc.vector.tensor_copy(u_next[:, :], un_ps[:, :])
                u_cur = u_next

            # ---------------- final step: last 256 columns ------------------
            ps4 = pp1.tile([1, OUT_N], F32, name="ps4", tag="aux", bufs=2)
            for k in range(KT):
                nc.tensor.matmul(
                    ps4[0:1, :],
                    lhsT=u_cur[:, k:k + 1],
                    rhs=wc[:, k * OUT_N:(k + 1) * OUT_N],
                    start=(k == 0),
                    stop=(k == KT - 1),
                )
            p4 = small.tile([1, OUT_N], F32, name="p4")
            nc.vector.tensor_copy(p4[0:1, :], ps4[0:1, :])
            cc4_in = dram.tile([1, OUT_N], F32, name="cc4_in")
            cc4_out = dram.tile([NCORES, SEG], F32, name="cc4_out")
            nc.gpsimd.dma_start(out=cc4_in[:, :], in_=p4[0:1, :])
            nc.gpsimd.collective_compute(
                "AllToAll",
                mybir.AluOpType.bypass,
                replica_groups=RG,
                ins=[cc4_in.opt()],
                outs=[cc4_out.opt()],
            )
            acc4 = small.tile([NCORES, SEG], F32, name="acc4")
            nc.sync.dma_start(out=acc4[0:NCORES, :], in_=cc4_out[:, :])
            v4_ps = pp1.tile([1, SEG], F32, name="v4_ps", tag="aux", bufs=2)
            nc.tensor.matmul(
                v4_ps[0:1, :],
                lhsT=ones8[0:NCORES, 0:1],
                rhs=acc4[0:NCORES, :],
                start=True,
                stop=True,
            )
            v4 = small.tile([1, SEG], F32, name="v4")
            nc.vector.tensor_copy(v4[0:1, :], v4_ps[0:1, :])
            res = small.tile([1, SEG], F32, name="res")
            nc.vector.tensor_mul(res[0:1, :], v4[0:1, :], dgt[0:1, :])
            nc.gpsimd.dma_start(out=out.ap(), in_=res[0:1, :])

    nc.compile()
    return nc


def _get(num_steps: int):
    if num_steps not in _cache:
        _cache[num_steps] = _build(num_steps)
    return _cache[num_steps]


# permuted Wr column order: position 4096h + 512j + t  <->  col 1024j + 512h + t
_PERM = np.concatenate(
    [np.arange(1024 * j + 512 * h, 1024 * j + 512 * h + 512)
     for h in (0, 1) for j in range(NCORES)]
)


def _shard_inputs(x: np.ndarray, W: np.ndarray):
    bf = ml_dtypes.bfloat16
    xT = np.ascontiguousarray(x[0].reshape(8, 128).T).astype(bf)
    dgv = np.ascontiguousarray(np.diagonal(W)[D0:]).astype(np.float32)
    in_maps = []
    for d in range(NCORES):
        blk = W[0:IN_N, RP * d: RP * (d + 1)]
        # column c of the device A must be W_block[:, 8p+m] for c = m*128+p
        A = np.ascontiguousarray(
            blk.reshape(IN_N, 128, 8).transpose(0, 2, 1).reshape(IN_N, RP)
        ).astype(bf)
        Wrd = W[RP * d: RP * (d + 1), :]
        # Wc tiled [k, p, c] with (k, p) <-> local row 8p+k
        Wc = np.ascontiguousarray(
            Wrd[:, D0:].reshape(128, 8, OUT_N).transpose(1, 0, 2)
        ).astype(bf)
        # Wr permuted then tiled to exact DMA order [j*8+k, p, c]
        Wr = np.ascontiguousarray(
            Wrd[:, _PERM].reshape(128, KT, N // PANEL, PANEL)
            .transpose(2, 1, 0, 3)
            .reshape(N // PANEL * KT, 128, PANEL)
        ).astype(bf)
        dg_d = np.ascontiguousarray(dgv[SEG * d: SEG * (d + 1)]).reshape(1, SEG)
        in_maps.append({"xT": xT, "A": A, "Wr": Wr, "Wc": Wc, "dg": dg_d})
    return in_maps


def _run(x, W, num_steps, trace=False):
    x = np.asarray(x, dtype=np.float32)
    W = np.asarray(W, dtype=np.float32)
    num_steps = int(num_steps)
    if num_steps == 0:
        # v0 is zero on the last 256 entries (x only fills the first 1024)
        return np.zeros(OUT_N, np.float32), None
    if num_steps == 1:
        # out = diag * v1[-256:]; tiny, never hit by the harness (4 steps)
        v1d = W[0:IN_N, D0:].T.astype(np.float64) @ x[0].astype(np.float64)
        return (np.diagonal(W)[D0:] * v1d).astype(np.float32), None
    nc = _get(num_steps)
    in_maps = _shard_inputs(x, W)
    r = run_bass_kernel_spmd(
        nc, in_maps, core_ids=list(range(NCORES)), trace=trace
    )
    outv = np.concatenate(
        [np.asarray(r.results[d]["out"], np.float32).reshape(SEG)
         for d in range(NCORES)]
    )
    return outv, r


def kernel(x, W, num_steps) -> np.ndarray:
    outv, _ = _run(x, W, num_steps, trace=False)
    return outv


def run_traced(x, W, num_steps):
    return _run(x, W, num_steps, trace=True)



# revision 3
# speedup vs baseline: 1.0134x; 1.0134x over previous
"""Trainium2 Bass kernel for nn_AdjacencyMatrix (gnn_message_passing) — v2.

Math (same reduction as before): the reference state is always
W * v[:, None], so the whole problem is the matvec chain
  a1 = W[0:1024,:]^T x,  a_{t+1} = W^T a_t,  out = diag(W)[-256:] * a_n[-256:].

v2 sharding: COLUMN-parallel.  Core d owns the column block
W[:, 1024d:1024d+1024] (bf16, SBUF-resident, streamed once from HBM).
 - a1[J_d] = W[0:1024, J_d]^T x       : local (k-tiles 0..7 of the shard)
 - middle steps: a_{t+1}[J_d] = W[:, J_d]^T a_t : local matvec, but needs the
   FULL a_t -> one AllGather of the 8KB bf16 vector per middle step.  The
   gathered vector is laid into partition-major [128, 64] with one xbar
   transpose-DMA, so matvec stationaries need no on-chip transposes.
 - last step: partial[t] = sum_{j in J_d} (W[j, D0+t]*diag[D0+t]) a3[j] :
   local against a small pre-scaled [1024, 256] tile, then one 1KB
   AllToAll + 8-way sum gives each core its 32 outputs (diag is folded into
   the tile on the host, so no final elementwise multiply).

PE matvecs use 4x column tiling (tile_position col groups 0/32/64/96, each
group owning a disjoint 256-col slice of the 1024 output columns), which
runs 4 matmuls concurrently: a full 8192x1024 matvec takes 64 waves of
~110ns instead of 27us.

Known wall: the first TOPSP collective pays a fixed ~48us CC rendezvous
barrier that starts ~21us into the NEFF and ends ~70us in, regardless of
when the collective is triggered.  The W stream (~46us) hides most of it;
the remaining tail is AG1 + a2 + AG2 + a3 + a4 + final A2A.
"""

import ml_dtypes
import numpy as np

import concourse.bass as bass
import concourse.mybir as mybir
from concourse import bacc, tile
from concourse.bass_utils import run_bass_kernel_spmd

N = 8192
IN_N = 1024
OUT_N = 256
NCORES = 8
CP = N // NCORES          # columns per core = 1024
KT = N // 128             # contract k-tiles for a full matvec = 64
D0 = N - OUT_N            # 7936
SEG = OUT_N // NCORES     # 32 outputs per core
NBLK = 8                  # W stream blocks (2 MiB each, 8 k-tiles)
KPB = KT // NBLK          # k-tiles per stream block = 8

F32 = mybir.dt.float32
BF16 = mybir.dt.bfloat16
RG = [list(range(NCORES))]

_cache: dict = {}


def _matvec_waves(nc, pout, u_sb, w_sb, nk, k0_tile=0, ucol0=0):
    """One full matvec pass: out[1, 1024] += sum_k u[:,k]^T @ W_ktile(k).

    4x col-tiled: group g computes output cols [256g, 256g+256) into the
    psum row at partition 32g.  nk contract k-tiles starting at k0_tile.
    """
    for k in range(nk):
        wbase = (k0_tile + k) * CP
        for g in range(4):
            nc.tensor.matmul(
                pout[32 * g:32 * g + 1, 0:256],
                lhsT=u_sb[:, ucol0 + k:ucol0 + k + 1],
                rhs=w_sb[:, wbase + 256 * g:wbase + 256 * (g + 1)],
                start=(k == 0),
                stop=(k == nk - 1),
                tile_position=(0, 32 * g),
            )


def _evac(nc, s_out, pin):
    """psum rows {0,32,64,96} x [1,256] -> sbuf [1, 1024] (casts)."""
    for g in range(4):
        eng = nc.vector.tensor_copy if g % 2 == 0 else nc.scalar.copy
        eng(out=s_out[0:1, 256 * g:256 * (g + 1)],
            in_=pin[32 * g:32 * g + 1, 0:256])


def _junk(nc, pj, js, u_sb, w_sb, n_mm):
    """PE warm block: n_mm matmuls into a junk psum bank + one fake use."""
    for i in range(n_mm):
        nc.tensor.matmul(
            pj[0:1, 0:512],
            lhsT=u_sb[:, 0:1],
            rhs=w_sb[:, 0:512],
            start=(i == 0),
            stop=(i == n_mm - 1),
        )
    nc.scalar.copy(out=js[0:1, :], in_=pj[0:1, 0:512])


def _build(num_steps: int):
    assert num_steps >= 2
    n_mid = num_steps - 2
    nc = bacc.Bacc(
        "TRN2", target_bir_lowering=False, debug=False, num_devices=NCORES
    )
    xT = nc.declare_dram_parameter("xT", [128, 8], BF16, isOutput=False)
    Wb = nc.declare_dram_parameter("Wb", [NBLK, 128, KPB * CP], BF16, isOutput=False)
    W4 = nc.declare_dram_parameter("W4", [128, 8 * OUT_N], BF16, isOutput=False)
    ident = nc.declare_dram_parameter("ident", [128, 128], BF16, isOutput=False)
    out = nc.declare_dram_parameter("out", [1, SEG], F32, isOutput=True)

    # collective buffers (DRAM)
    cc_ins = [
        nc.dram_tensor(f"cc{m}_in", [1, 1024], BF16) for m in range(n_mid + 1)
    ]
    gaths = [
        nc.dram_tensor(f"G{m}", [64, 128], BF16, addr_space="Shared")
        for m in range(n_mid)
    ]
    cc4_in = nc.dram_tensor("cc4_in", [1, OUT_N], F32)
    cc4_out = nc.dram_tensor("cc4_out", [NCORES, SEG], F32)

    with tile.TileContext(nc) as tc:
        with (
            tc.tile_pool(name="small", bufs=1) as small,
            tc.tile_pool(name="wres", bufs=1) as wres,
            tc.tile_pool(name="ppool", bufs=1, space="PSUM") as ppool,
        ):
            # ---- small loads (scalar HWDGE ring) ----
            xt = small.tile([128, 8], BF16, name="xt")
            nc.scalar.dma_start(out=xt[:, :], in_=xT.ap())
            w4 = small.tile([128, 8 * OUT_N], BF16, name="w4")
            nc.scalar.dma_start(out=w4[:, :], in_=W4.ap())
            idt = small.tile([128, 128], BF16, name="idt")
            nc.scalar.dma_start(out=idt[:, :], in_=ident.ap())
            ones8 = small.tile([8, 1], F32, name="ones8")
            nc.vector.memset(ones8[0:8, :], 1.0)

            # ---- W stream: 8 x 2 MiB blocks on the sync HWDGE ring ----
            wk = wres.tile([128, KT * CP], BF16, name="wk")
            for b in range(NBLK):
                nc.sync.dma_start(
                    out=wk[:, b * KPB * CP:(b + 1) * KPB * CP],
                    in_=Wb.ap()[b],
                )

            # psum banks
            pA = ppool.tile([128, 512], F32, name="pA")
            pB = [ppool.tile([128, 512], F32, name=f"pB{m}") for m in range(n_mid)]
            pD = ppool.tile([128, 512], F32, name="pD")
            pJ = ppool.tile([128, 512], F32, name="pJ")
            pv = ppool.tile([1, SEG], F32, name="pv")
            pT = ppool.tile([128, 16], BF16, name="pT")
            js = small.tile([1, 512], F32, name="js")

            # ---- a1: local, k-tiles 0..7 (rows 0..1023) ----
            _matvec_waves(nc, pA, xt, wk, nk=8, k0_tile=0, ucol0=0)
            s_cur = small.tile([1, 1024], BF16, name="s1")
            _evac(nc, s_cur, pA)

            # ---- middle steps: AllGather -> transpose-DMA -> matvec ----
            for m in range(n_mid):
                nc.scalar.dma_start(out=cc_ins[m].ap(), in_=s_cur[0:1, :])
                nc.gpsimd.collective_compute(
                    "AllGather", mybir.AluOpType.bypass, replica_groups=RG,
                    ins=[cc_ins[m].ap()], outs=[gaths[m].ap()],
                )
                u_sb = small.tile([128, KT], BF16, name=f"u{m + 2}")
                nc.scalar.dma_start(out=u_sb[:, :], in_=gaths[m].ap(), transpose=True)
                _matvec_waves(nc, pB[m], u_sb, wk, nk=KT)
                s_cur = small.tile([1, 1024], BF16, name=f"s{m + 2}")
                _evac(nc, s_cur, pB[m])

            # ---- last step: local vs pre-scaled W4, then 1KB A2A + sum ----
            # transpose s_cur [1, 1024] -> u4 [128, 8] partition-major
            u4 = small.tile([128, 16], BF16, name="u4")
            for kl in range(8):
                nc.tensor.transpose(
                    pT[0:128, 2 * kl:2 * kl + 1],
                    s_cur[0:1, 128 * kl:128 * (kl + 1)],
                    idt[0:1, 0:1],
                )
            nc.vector.tensor_copy(u4[:, :], pT[0:128, 0:16])
            for kl in range(8):
                nc.tensor.matmul(
                    pD[0:1, 0:OUT_N],
                    lhsT=u4[:, 2 * kl:2 * kl + 1],
                    rhs=w4[:, OUT_N * kl:OUT_N * (kl + 1)],
                    start=(kl == 0),
                    stop=(kl == 7),
                )
            s4 = small.tile([1, OUT_N], F32, name="s4")
            nc.scalar.copy(out=s4[0:1, :], in_=pD[0:1, 0:OUT_N])
            nc.scalar.dma_start(out=cc4_in.ap(), in_=s4[0:1, :])
            nc.gpsimd.collective_compute(
                "AllToAll", mybir.AluOpType.bypass, replica_groups=RG,
                ins=[cc4_in.ap()], outs=[cc4_out.ap()],
            )
            acc4 = small.tile([NCORES, SEG], F32, name="acc4")
            nc.scalar.dma_start(out=acc4[0:NCORES, :], in_=cc4_out.ap())
            nc.tensor.matmul(
                pv[0:1, :],
                lhsT=ones8[0:NCORES, 0:1],
                rhs=acc4[0:NCORES, :],
                start=True,
                stop=True,
            )
            res = small.tile([1, SEG], F32, name="res")
            nc.vector.tensor_copy(res[0:1, :], pv[0:1, :])
            nc.scalar.dma_start(out=out.ap(), in_=res[0:1, :])

    nc.compile()
    return nc


def _get(num_steps: int):
    if num_steps not in _cache:
        _cache[num_steps] = _build(num_steps)
    return _cache[num_steps]


def _shard_inputs(x: np.ndarray, W: np.ndarray):
    bf = ml_dtypes.bfloat16
    xT = np.ascontiguousarray(x[0].reshape(8, 128).T).astype(bf)
    dgv = np.diagonal(W)[D0:].astype(np.float32)
    idn = np.eye(128, dtype=np.float32).astype(bf)
    in_maps = []
    for d in range(NCORES):
        Wd = W[:, CP * d:CP * (d + 1)]
        # stream blocks: [b, p, (kl, c)] = Wd[128*(8b + kl) + p, c]
        Wb = np.ascontiguousarray(
            Wd.reshape(NBLK, KPB, 128, CP).transpose(0, 2, 1, 3)
            .reshape(NBLK, 128, KPB * CP)
        ).astype(bf)
        # last-step tile, diag pre-folded: [p, (kl, t)] =
        #   Wd4[128*kl + p, t] * diag[D0 + t]
        Wd4 = W[CP * d:CP * (d + 1), D0:] * dgv[None, :]
        W4 = np.ascontiguousarray(
            Wd4.reshape(8, 128, OUT_N).transpose(1, 0, 2).reshape(128, 8 * OUT_N)
        ).astype(bf)
        in_maps.append({"xT": xT, "Wb": Wb, "W4": W4, "ident": idn})
    return in_maps


def _run(x, W, num_steps, trace=False):
    x = np.asarray(x, dtype=np.float32)
    W = np.asarray(W, dtype=np.float32)
    num_steps = int(num_steps)
    if num_steps == 0:
        return np.zeros(OUT_N, np.float32), None
    if num_steps == 1:
        v1d = W[0:IN_N, D0:].T.astype(np.float64) @ x[0].astype(np.float64)
        return (np.diagonal(W)[D0:] * v1d).astype(np.float32), None
    nc = _get(num_steps)
    in_maps = _shard_inputs(x, W)
    r = run_bass_kernel_spmd(
        nc, in_maps, core_ids=list(range(NCORES)), trace=trace
    )
    outv = np.concatenate(
        [np.asarray(r.results[d]["out"], np.float32).reshape(SEG)
         for d in range(NCORES)]
    )
    return outv, r


def kernel(x, W, num_steps) -> np.ndarray:
    outv, _ = _run(x, W, num_steps, trace=False)
    return outv


def run_traced(x, W, num_steps):
    return _run(x, W, num_steps, trace=True)
